# revision 2
# baseline (speedup 1.0000x reference)
"""Trainium2 Bass kernel: batched multi-head attention with per-frame
conditioning K/V token (nn_Attention dense_transformer problem).

Data-parallel over the 16 (b*n) frames -> 2 frames per NeuronCore, no
collectives. Per core, a fused kernel: QKV projection (q,k feature-major;
v token-major) -> per-head attention with sim computed transposed (keys on
partitions) so softmax denominators come from a ones-column in the PV
matmul -> output projection.

Scheduling (the main speedup over v1): software-pipelined emission.
Attention blocks (sim -> exp -> pv) are the scalar-engine-feeding backbone;
all other PE work (QKV projection chunks, v emission, cond-token k/v,
out-projection) is queued as thunks with deadlines and injected between jc
iterations, so ACT never starves and PE never bursts. Within a block, sims
run one jc ahead of pvs (a pv waiting on its exp would head-of-line-block
the PE wait queue, depth 4); the last pv spills into the next block; the
reciprocal/normalize epilogue is deferred into the next block so its DVE
latency hides behind sims. One exp per (pair, jc) covers both heads via a
2-bank PSUM tile. Input DMAs are strided and ordered by first use; output
is DMA'd as bf16 (host casts back to f32).

Layout notes:
 - All matmul operands bf16 (f32 PSUM accumulation). Host pre-transposes x
   to feature-major and pre-splits d into 128-row chunks, so no on-device
   transposes are needed anywhere.
 - Keys padded: col T = conditioning token, cols T+1..T+127 zero dummies.
   The dummy keys' v rows AND ones-column entries are zero, so they add
   nothing to the attention output or the softmax denominator.
 - v stored interleaved [8 heads x 72 cols] (64 v + ones-col at 64 + 7 pad)
   so each head's PV stationary operand is a contiguous (128, 65) slice at
   a 16B-aligned offset (HW weight requirement); PV output row 64
   accumulates the softmax denominator for free.
"""

import numpy as np
import ml_dtypes

import concourse.bacc as bacc
import concourse.tile as tile
from concourse import mybir
from concourse.bass_utils import run_bass_kernel_spmd

BF16 = mybir.dt.bfloat16
F32 = mybir.dt.float32

HEADS = 8
DH = 64
D = 512
HID = 512
SCALE = DH ** -0.5
N_CORES = 8
NDC = D // 128  # 4 contraction chunks of 128


def build_attention_nc(T=1024, loop_n=1):
    S = T + 128             # keys T, cond at col T, 127 zero dummies
    JC = S // 128           # key chunks (9 for T=1024)
    NI = min(512, T)        # i-tile width (matmul moving free dim)
    NIH = T // NI           # i-tiles per frame
    NTC = T // 128          # token chunks (for v / out-proj)

    nc = bacc.Bacc("TRN2", target_bir_lowering=False)
    x_d = nc.declare_dram_parameter("xT", [128, NDC, 2, T], BF16, isOutput=False)
    w_d = nc.declare_dram_parameter("Wqkv", [128, NDC, 3 * HID], BF16, isOutput=False)
    wk_d = nc.declare_dram_parameter("Wk", [128, NDC, HID], BF16, isOutput=False)
    wv_d = nc.declare_dram_parameter("Wv", [128, NDC, HID], BF16, isOutput=False)
    wo_d = nc.declare_dram_parameter("Wout", [128, NDC, D], BF16, isOutput=False)
    lab_d = nc.declare_dram_parameter("labT", [128, NDC, 2, 8], BF16, isOutput=False)
    f_d = nc.declare_dram_parameter("F", [33, 128], BF16, isOutput=False)
    out_d = nc.declare_dram_parameter("out", [2, T, D], BF16, isOutput=True)

    EXP = mybir.ActivationFunctionType.Exp

    with tile.TileContext(nc) as tc:
        with (
            tc.tile_pool(name="persist", bufs=1) as pp,
            tc.tile_pool(name="work", bufs=8) as wp,
            tc.tile_pool(name="psum", bufs=1, space="PSUM") as psp,
        ):
            def emit_body():
                # ---- persistent SBUF tiles ----
                xT = pp.tile([128, NDC, 2, T], BF16, tag="xT")
                wq = pp.tile([128, NDC, 3 * HID], BF16, tag="wq")
                wk = pp.tile([128, NDC, HID], BF16, tag="wk")
                wv = pp.tile([128, NDC, HID], BF16, tag="wv")
                wo = pp.tile([128, NDC, D], BF16, tag="wo")
                lab = pp.tile([128, NDC, 2, 8], BF16, tag="lab")
                qT = pp.tile([128, NDC, 2, T], BF16, tag="qT")
                kT = pp.tile([128, NDC, 2, S], BF16, tag="kT")
                # 72*2B = 144B: 16B-aligned per-head stride (HW weight req)
                vv = pp.tile([128, 2, JC, HEADS, 72], BF16, tag="vv")
                attn = pp.tile([128, NDC, 2, T], BF16, tag="attn")
                fmat = pp.tile([33, 128], BF16, tag="fmat")
                # 1/denom rows: 0 (h1) and 32 (h2); rows 1-31 stay 1.0
                rg = pp.tile([33, NI], BF16, tag="rg")

                # ---- input DMAs, strided, ordered by first use ----
                nc.sync.dma_start(wq[:, :, 0:128], w_d[:, :, 0:128])      # q cc0
                nc.sync.dma_start(wq[:, :, 512:640], w_d[:, :, 512:640])  # k cc4
                nc.sync.dma_start(xT[:, :, 0, 0:NI], x_d[:, :, 0, 0:NI])
                nc.sync.dma_start(wq[:, :, 2 * HID:3 * HID],              # v cols
                                  w_d[:, :, 2 * HID:3 * HID])
                nc.sync.dma_start(xT[:, :, 0, NI:T], x_d[:, :, 0, NI:T])
                nc.sync.dma_start(wq[:, :, 128:512], w_d[:, :, 128:512])
                nc.sync.dma_start(wq[:, :, 640:1024], w_d[:, :, 640:1024])
                nc.sync.dma_start(lab[:], lab_d[:])
                nc.sync.dma_start(wk[:], wk_d[:])
                nc.sync.dma_start(wv[:], wv_d[:])
                nc.sync.dma_start(fmat[:], f_d[:])
                for dc in range(NDC):  # frame-1 activations
                    nc.sync.dma_start(xT[:, dc, 1], x_d[:, dc, 1])
                nc.sync.dma_start(wo[:], wo_d[:])

                # constants / padding init (rg rows 1-31 finite; F rows 0 there)
                nc.vector.memset(rg[:], 1.0)
                nc.vector.memset(kT[:, :, :, T + 1:S], 0.0)  # dummy keys
                nc.vector.memset(vv[:, :, JC - 1, :, 0:65], 0.0)  # dummy v + ones col
                nc.vector.memset(vv[:, :, 0:JC - 1, :, DH:DH + 1], 1.0)  # ones (real)
                nc.vector.memset(vv[0:1, :, JC - 1, :, DH:DH + 1], 1.0)  # cond token

                # ---- emission helpers (each returns a list of thunks) ----
                def v_chunk(f, tc_i):
                    def t():
                        ps = psp.tile([128, 2, NI], F32, tag="sim", bufs=2)
                        for dc in range(NDC):
                            nc.tensor.matmul(
                                ps[:, 0, 0:HID],
                                xT[:, dc, f, tc_i * 128:(tc_i + 1) * 128],
                                wq[:, dc, 2 * HID:3 * HID],
                                start=(dc == 0), stop=(dc == NDC - 1),
                            )
                        nc.vector.tensor_copy(vv[:, f, tc_i, :, 0:DH], ps[:, 0, 0:HID])
                    return t

                def ek_thunk():
                    def t():
                        for cc in range(NDC):
                            ps = psp.tile([128, 2, NI], F32, tag="sim", bufs=2)
                            for dc in range(NDC):
                                nc.tensor.matmul(
                                    ps[:, 0, 0:2],
                                    wk[:, dc, cc * 128:(cc + 1) * 128],
                                    lab[:, dc, :, 0:1],
                                    start=(dc == 0), stop=(dc == NDC - 1),
                                )
                            for f in range(2):
                                nc.vector.tensor_copy(
                                    kT[:, cc, f, T:T + 1], ps[:, 0, f:f + 1])
                    return t

                def ev_thunk(f):
                    def t():
                        ps = psp.tile([128, 2, NI], F32, tag="sim", bufs=2)
                        for dc in range(NDC):
                            nc.tensor.matmul(
                                ps[0:1, 0, 0:HID],
                                lab[:, dc, f, 0:1],
                                wv[:, dc, :],
                                start=(dc == 0), stop=(dc == NDC - 1),
                            )
                        nc.vector.tensor_copy(vv[0:1, f, JC - 1, :, 0:DH],
                                              ps[0:1, 0, 0:HID])
                    return t

                def qk_chunk(f, cc, ih):
                    def t():
                        ps = psp.tile([128, 2, NI], F32, tag="sim", bufs=2)
                        for dc in range(NDC):
                            nc.tensor.matmul(
                                ps[:, 0],
                                wq[:, dc, cc * 128:(cc + 1) * 128],
                                xT[:, dc, f, ih * NI:(ih + 1) * NI],
                                start=(dc == 0), stop=(dc == NDC - 1),
                            )
                        if cc < 4:
                            nc.vector.tensor_copy(qT[:, cc, f, ih * NI:(ih + 1) * NI],
                                                  ps[:, 0])
                        else:
                            nc.vector.tensor_copy(kT[:, cc - 4, f, ih * NI:(ih + 1) * NI],
                                                  ps[:, 0])
                    return t

                def proj_chunk(f, ic):
                    def t():
                        ps = psp.tile([128, 2, NI], F32, tag="sim", bufs=2)
                        for a in range(NDC):
                            nc.tensor.matmul(
                                ps[:, 0],
                                attn[:, a, f, ic * 128:(ic + 1) * 128],
                                wo[:, a, :],
                                start=(a == 0), stop=(a == NDC - 1),
                            )
                        ot = wp.tile([128, D], BF16, tag="oout")
                        nc.vector.tensor_copy(ot[:], ps[:, 0])
                        nc.sync.dma_start(out_d[f, ic * 128:(ic + 1) * 128, :], ot[:])
                    return t

                # ---- the injection queue ----
                # entries: (due_slot, earliest_slot, thunk). A thunk is
                # force-emitted when its deadline nears; otherwise one
                # optional thunk runs every other slot to spread PE load.
                queue = []
                cur_slot = [0]

                def inject():
                    cur = cur_slot[0]
                    while queue and queue[0][0] <= cur + 3:
                        queue.pop(0)[2]()
                    if queue and queue[0][1] <= cur:
                        queue.pop(0)[2]()

                def drain():
                    while queue:
                        queue.pop(0)[2]()

                def emit_block(f, a, ih, epi_prev=None):
                    isl = slice(ih * NI, (ih + 1) * NI)
                    pvA = psp.tile([65, NI], F32, tag="pv", bufs=4)
                    pvB = psp.tile([65, NI], F32, tag="pv", bufs=4)
                    # software-pipelined: sims run one jc ahead of pvs so a
                    # pv waiting on its exp never head-of-line-blocks the
                    # next sims in the PE queue
                    pABs = [None] * JC

                    def emit_sim(jc):
                        jsl = slice(jc * 128, (jc + 1) * 128)
                        sAB = psp.tile([128, 2, NI], F32, tag="sim", bufs=2)
                        nc.tensor.matmul(
                            sAB[:, 0], kT[0:64, a, f, jsl], qT[0:64, a, f, isl],
                            start=True, stop=True, tile_position=(0, 0),
                        )
                        nc.tensor.matmul(
                            sAB[:, 1], kT[64:128, a, f, jsl], qT[64:128, a, f, isl],
                            start=True, stop=True, tile_position=(64, 0),
                        )
                        # one exp covering both heads' chunks (2 PSUM banks)
                        pAB = wp.tile([128, 2, NI], BF16, tag="P")
                        nc.scalar.activation(pAB[:], sAB[:], EXP, scale=SCALE)
                        pABs[jc] = pAB

                    def emit_pv(jc):
                        nc.tensor.matmul(
                            pvA[:], vv[:, f, jc, 2 * a, 0:65], pABs[jc][:, 0],
                            start=(jc == 0), stop=(jc == JC - 1),
                        )
                        nc.tensor.matmul(
                            pvB[:], vv[:, f, jc, 2 * a + 1, 0:65], pABs[jc][:, 1],
                            start=(jc == 0), stop=(jc == JC - 1),
                        )

                    for jc in range(JC):
                        emit_sim(jc)
                        if jc == 0:
                            # previous block's last pv spills into this block
                            # so it never blocks our first sims
                            if pv_spill:
                                pv_spill.pop(0)()
                        else:
                            emit_pv(jc - 1)
                        if jc == 1 and epi_prev is not None:
                            # previous block's reciprocals land on DVE now,
                            # their latency hidden behind this block's sims
                            epi_prev[0]()
                        elif jc == 3 and epi_prev is not None:
                            epi_prev[1]()
                        inject()
                        cur_slot[0] += 1
                    pv_spill.append(lambda: emit_pv(JC - 1))

                    def epi_recip():
                        # reciprocal softmax denominators (dummies contribute 0)
                        with nc.allow_low_precision("softmax denom recip bf16"):
                            nc.vector.reciprocal(rg[0:1, :], pvA[64:65, :])
                            nc.vector.reciprocal(rg[32:33, :], pvB[64:65, :])

                    def epi_norm():
                        # broadcast 1/denom: h1 -> partitions 0:64, h2 -> 64:128
                        bc = psp.tile([128, 2, NI], F32, tag="sim", bufs=2)
                        nc.tensor.matmul(bc[:, 0], fmat[:], rg[:], start=True, stop=True)
                        rbc = wp.tile([128, NI], BF16, tag="rbc")
                        nc.vector.tensor_copy(rbc[:], bc[:, 0])
                        nc.vector.tensor_mul(attn[0:64, a, f, isl], pvA[0:64, :], rbc[0:64, :])
                        nc.vector.tensor_mul(attn[64:128, a, f, isl], pvB[0:64, :], rbc[64:128, :])

                    return (epi_recip, epi_norm)

                # ---- schedule ----
                # prologue: minimum to start block (0, 0, 0)
                qk_chunk(0, 0, 0)(); qk_chunk(0, 4, 0)()
                v_chunk(0, 0)(); v_chunk(0, 1)()

                def slot_of(bi, jc):
                    return bi * JC + jc

                def fill_frame_queue(f):
                    b0 = 8 * f
                    ents = []
                    # v chunks: consumer pv(i) of block b0 at slot b0*9+i+1
                    for i in range(2 if f == 0 else 0, NTC):
                        ents.append((slot_of(b0, max(i - 1, 0)), 0, v_chunk(f, i)))
                    # k chunks: sims jc in [4ih, 4ih+3] of pair blocks
                    # q chunks: sims of block (f, p, ih)
                    for p in range(1, 4):
                        for ih in range(NIH):
                            ents.append((slot_of(b0 + 2 * p, 4 * ih) - 1, 0,
                                         qk_chunk(f, p + 4, ih)))
                            ents.append((slot_of(b0 + 2 * p + ih, 0) - 1, 0,
                                         qk_chunk(f, p, ih)))
                    if f == 0:
                        # pair-0 second halves (ih=1) not covered by prologue
                        ents.append((slot_of(0, 4) - 1, 0, qk_chunk(0, 4, 1)))
                        ents.append((slot_of(1, 0) - 1, 0, qk_chunk(0, 0, 1)))
                        ents.append((slot_of(0, 7), 0, ek_thunk()))
                        # frame-1 head start (pair 0 q/k)
                        for ih in range(NIH):
                            ents.append((slot_of(8, 4 * ih) - 1, 0,
                                         qk_chunk(1, 4, ih)))
                            ents.append((slot_of(8 + ih, 0) - 1, 0,
                                         qk_chunk(1, 0, ih)))
                    ents.append((slot_of(b0, 8), 0, ev_thunk(f)))
                    if f == 1:
                        # frame-0 out-projection: attn(f=0) complete after
                        # block-7's epilogue, which runs early in block 8
                        for ic in range(NTC):
                            ents.append((slot_of(15, 8), slot_of(8, 4) + 2 * ic,
                                         proj_chunk(0, ic)))
                    queue.extend(sorted(ents, key=lambda e: e[0]))

                fill_frame_queue(0)
                blocks = [(f, a, ih) for f in range(2) for a in range(4)
                          for ih in range(NIH)]
                epi = None
                pv_spill = []
                for bi, (f, a, ih) in enumerate(blocks):
                    if bi == 4:
                        fill_frame_queue(1)
                    if bi == 15:
                        # frame-1 ih=0 projection: deps ready once block-14's
                        # epilogue (emitted at jc=3 of this block) is done
                        queue.extend((slot_of(15, 8), slot_of(15, 4) + ic, proj_chunk(1, ic))
                                     for ic in range(NTC // 2))
                    epi = emit_block(f, a, ih, epi_prev=epi)
                while pv_spill:
                    pv_spill.pop(0)()
                epi[0](); epi[1]()
                drain()
                # epilogue: frame-1 second-half projection
                for ic in range(NTC // 2, NTC):
                    proj_chunk(1, ic)()

            if loop_n > 1:
                with tc.For_i(0, loop_n, 1):
                    emit_body()
            else:
                emit_body()

    nc.finalize()
    return nc


_NC_CACHE = {}


def _get_nc(T):
    if T not in _NC_CACHE:
        _NC_CACHE[T] = build_attention_nc(T)
    return _NC_CACHE[T]


def make_in_maps(x, label_emb_mm, Wqkv, Wk, Wv, Wout):
    """Host-side sharding + layout prep (transpose to feature-major, bf16)."""
    bf = ml_dtypes.bfloat16
    BN, T, d = x.shape
    assert (BN, d) == (16, D)
    # x[fr, t, dc*128+p] -> xB[fr, p, dc, t]
    xB = np.ascontiguousarray(
        np.asarray(x).reshape(16, T, NDC, 128).transpose(0, 3, 2, 1)
    ).astype(bf)
    wq = np.ascontiguousarray(np.asarray(Wqkv).reshape(NDC, 128, 3 * HID).transpose(1, 0, 2)).astype(bf)
    wkh = np.ascontiguousarray(np.asarray(Wk).reshape(NDC, 128, HID).transpose(1, 0, 2)).astype(bf)
    wvh = np.ascontiguousarray(np.asarray(Wv).reshape(NDC, 128, HID).transpose(1, 0, 2)).astype(bf)
    woh = np.ascontiguousarray(np.asarray(Wout).reshape(NDC, 128, D).transpose(1, 0, 2)).astype(bf)
    labB = np.asarray(label_emb_mm).reshape(16, NDC, 128)  # [fr, dc, p]
    F = np.zeros((33, 128), dtype=bf)
    F[0, 0:64] = 1.0
    F[32, 64:128] = 1.0
    in_maps = []
    for c in range(N_CORES):
        xTc = np.ascontiguousarray(xB[2 * c:2 * c + 2].transpose(1, 2, 0, 3))  # (128,4,2,T)
        labc2 = np.ascontiguousarray(labB[2 * c:2 * c + 2].transpose(2, 1, 0)).astype(bf)  # (128,4,2)
        labc = np.zeros((128, NDC, 2, 8), dtype=bf)  # padded so f-stride is 16B
        labc[:, :, :, 0] = labc2
        in_maps.append({
            "xT": xTc, "Wqkv": wq, "Wk": wkh, "Wv": wvh, "Wout": woh, "labT": labc,
            "F": F,
        })
    return in_maps


def kernel(x, label_emb_mm, Wqkv, Wk, Wv, Wout, b):
    x = np.asarray(x)
    T = x.shape[1]
    nc = _get_nc(T)
    in_maps = make_in_maps(x, label_emb_mm, Wqkv, Wk, Wv, Wout)
    res = run_bass_kernel_spmd(nc, in_maps, core_ids=list(range(N_CORES)))
    out = np.concatenate([res.results[c]["out"] for c in range(N_CORES)], axis=0)
    return np.ascontiguousarray(out.reshape(16, T, D)).astype(np.float32)


# revision 3
# speedup vs baseline: 1.2375x; 1.2375x over previous
"""Trainium2 Bass kernel: batched multi-head attention with per-frame
conditioning K/V token (nn_Attention dense_transformer problem).

Data-parallel over the 16 (b*n) frames -> 2 frames per NeuronCore, no
collectives. Per core, a fused kernel: QKV projection (q,k feature-major;
v token-major) -> per-head attention with sim computed transposed (keys on
partitions) so softmax denominators come from a ones-column in the PV
matmul -> output projection.

Scheduling (the main speedup over v1): software-pipelined emission.
Attention blocks (sim -> exp -> pv) are the scalar-engine-feeding backbone;
all other PE work (QKV projection chunks, v emission, cond-token k/v,
out-projection) is queued as thunks with deadlines and injected between jc
iterations, so ACT never starves and PE never bursts. Within a block, sims
run two jc ahead of pvs (a pv waiting on its exp would head-of-line-block
the PE wait queue, depth 4); the last pv and the cond-token pv spill into
the next block; the reciprocal/normalize epilogue is deferred into the next
block so its DVE latency hides behind sims. One exp per (pair, jc) covers
both heads via a 2-bank PSUM tile. Input DMAs are strided and ordered by
first use; output is DMA'd as bf16 (host casts back to f32).

Conditioning token: instead of padding keys to T+128 (which costs a full
extra key-chunk of sim/exp/pv per block), the cond token runs through its
own block-diagonal path: ekk holds [ekA | ekB] key columns at rows 0/32 of
a (128, 33) stationary, giving both heads' cond logits in one matmul; a
small (33, NI) exp produces pc; and a rank-1 [ev_h | 1] x pc matmul closes
each pv accumulation (the ones entry adds exp(cond) to the denominator).

Layout notes:
 - All matmul operands bf16 (f32 PSUM accumulation). Host pre-transposes x
   to feature-major and pre-splits d into 128-row chunks, so no on-device
   transposes are needed anywhere.
 - v stored interleaved [8 heads x 72 cols] (64 v + ones-col at 64 + 7 pad)
   so each head's PV stationary operand is a contiguous (128, 65) slice at
   a 16B-aligned offset (HW weight requirement); PV output row 64
   accumulates the softmax denominator for free.
"""

import numpy as np
import ml_dtypes

import concourse.bacc as bacc
import concourse.tile as tile
from concourse import mybir
from concourse.bass_utils import run_bass_kernel_spmd

BF16 = mybir.dt.bfloat16
F32 = mybir.dt.float32

HEADS = 8
DH = 64
D = 512
HID = 512
SCALE = DH ** -0.5
N_CORES = 8
NDC = D // 128  # 4 contraction chunks of 128


def build_attention_nc(T=1024, loop_n=1):
    S = T                   # keys only; cond token handled separately
    JC = S // 128           # key chunks (8 for T=1024)
    NI = min(512, T)        # i-tile width (matmul moving free dim)
    NIH = T // NI           # i-tiles per frame
    NTC = T // 128          # token chunks (for v / out-proj)

    nc = bacc.Bacc("TRN2", target_bir_lowering=False)
    x_d = nc.declare_dram_parameter("xT", [128, NDC, 2, T], BF16, isOutput=False)
    w_d = nc.declare_dram_parameter("Wqkv", [128, NDC, 3 * HID], BF16, isOutput=False)
    wk_d = nc.declare_dram_parameter("Wk", [128, NDC, HID], BF16, isOutput=False)
    wv_d = nc.declare_dram_parameter("Wv", [128, NDC, HID], BF16, isOutput=False)
    wo_d = nc.declare_dram_parameter("Wout", [128, NDC, D], BF16, isOutput=False)
    lab_d = nc.declare_dram_parameter("labT", [128, NDC, 2, 40], BF16, isOutput=False)
    f_d = nc.declare_dram_parameter("F", [33, 128], BF16, isOutput=False)
    out_d = nc.declare_dram_parameter("out", [2, T, D], BF16, isOutput=True)

    EXP = mybir.ActivationFunctionType.Exp

    with tile.TileContext(nc) as tc:
        with (
            tc.tile_pool(name="persist", bufs=1) as pp,
            tc.tile_pool(name="work", bufs=8) as wp,
            tc.tile_pool(name="psum", bufs=1, space="PSUM") as psp,
        ):
            def emit_body():
                # ---- persistent SBUF tiles ----
                xT = pp.tile([128, NDC, 2, T], BF16, tag="xT")
                wq = pp.tile([128, NDC, 3 * HID], BF16, tag="wq")
                wk = pp.tile([128, NDC, HID], BF16, tag="wk")
                wv = pp.tile([128, NDC, HID], BF16, tag="wv")
                wo = pp.tile([128, NDC, D], BF16, tag="wo")
                lab = pp.tile([128, NDC, 2, 40], BF16, tag="lab")
                qT = pp.tile([128, NDC, 2, T], BF16, tag="qT")
                kT = pp.tile([128, NDC, 2, S], BF16, tag="kT")
                # 72*2B = 144B: 16B-aligned per-head stride (HW weight req)
                vv = pp.tile([128, 2, JC, HEADS, 72], BF16, tag="vv")
                # cond-token stationaries: ekk block-diag key cols (rows 0/32)
                # per (f, pair); evp rows [ev_h | 1] for the cond rank-1 pv
                ekk = pp.tile([128, 2, 4, 40], BF16, tag="ekk")
                evp = pp.tile([33, 2, 4, 72], BF16, tag="evp")
                attn = pp.tile([128, NDC, 2, T], BF16, tag="attn")
                fmat = pp.tile([33, 128], BF16, tag="fmat")
                # 1/denom rows: 0 (h1) and 32 (h2); rows 1-31 stay 1.0
                rg = pp.tile([33, NI], BF16, tag="rg")

                # ---- input DMAs, strided, ordered by first use ----
                nc.sync.dma_start(wq[:, :, 0:128], w_d[:, :, 0:128])      # q cc0
                nc.sync.dma_start(wq[:, :, 512:640], w_d[:, :, 512:640])  # k cc4
                nc.sync.dma_start(xT[:, :, 0, 0:NI], x_d[:, :, 0, 0:NI])
                nc.sync.dma_start(wq[:, :, 2 * HID:3 * HID],              # v cols
                                  w_d[:, :, 2 * HID:3 * HID])
                nc.sync.dma_start(xT[:, :, 0, NI:T], x_d[:, :, 0, NI:T])
                nc.sync.dma_start(wq[:, :, 128:512], w_d[:, :, 128:512])
                nc.sync.dma_start(wq[:, :, 640:1024], w_d[:, :, 640:1024])
                nc.sync.dma_start(lab[:], lab_d[:])
                nc.sync.dma_start(wk[:], wk_d[:])
                nc.sync.dma_start(wv[:], wv_d[:])
                nc.sync.dma_start(fmat[:], f_d[:])
                for dc in range(NDC):  # frame-1 activations
                    nc.sync.dma_start(xT[:, dc, 1], x_d[:, dc, 1])
                nc.sync.dma_start(wo[:], wo_d[:])

                # constants init (rg rows 1-31 finite; F rows 0 there)
                nc.vector.memset(rg[:], 1.0)
                nc.vector.memset(vv[:, :, :, :, DH:DH + 1], 1.0)  # ones col
                nc.vector.memset(ekk[:], 0.0)
                nc.vector.memset(evp[:, :, :, DH:DH + 1], 1.0)  # cond ones

                # ---- emission helpers (each returns a list of thunks) ----
                def v_chunk(f, tc_i):
                    def t():
                        ps = psp.tile([128, 2, NI], F32, tag="sim", bufs=2)
                        for dc in range(NDC):
                            nc.tensor.matmul(
                                ps[:, 0, 0:HID],
                                xT[:, dc, f, tc_i * 128:(tc_i + 1) * 128],
                                wq[:, dc, 2 * HID:3 * HID],
                                start=(dc == 0), stop=(dc == NDC - 1),
                            )
                        nc.vector.tensor_copy(vv[:, f, tc_i, :, 0:DH], ps[:, 0, 0:HID])
                    return t

                def ekk_thunk():
                    def t():
                        # ek per pair: (128 feats, 2 frames) -> block-diag
                        # stationaries: A at col 0 rows 0:64, B col 32 rows 64:128
                        for cc in range(NDC):
                            ps = psp.tile([128, 2, NI], F32, tag="sim", bufs=2)
                            for dc in range(NDC):
                                nc.tensor.matmul(
                                    ps[:, 0, 0:2],
                                    wk[:, dc, cc * 128:(cc + 1) * 128],
                                    lab[:, dc, :, 0:1],
                                    start=(dc == 0), stop=(dc == NDC - 1),
                                )
                            for f in range(2):
                                nc.vector.tensor_copy(
                                    ekk[0:64, f, cc, 0:1], ps[0:64, 0, f:f + 1])
                                nc.vector.tensor_copy(
                                    ekk[64:128, f, cc, 32:33], ps[64:128, 0, f:f + 1])
                    return t

                def evp_thunk(f):
                    def t():
                        # ev on partitions 0 AND 32 (lab col duplicated on host)
                        ps = psp.tile([128, 2, NI], F32, tag="sim", bufs=2)
                        for dc in range(NDC):
                            nc.tensor.matmul(
                                ps[0:33, 0, 0:HID],
                                lab[:, dc, f, 0:33],
                                wv[:, dc, :],
                                start=(dc == 0), stop=(dc == NDC - 1),
                            )
                        for a2 in range(4):
                            nc.vector.tensor_copy(
                                evp[0:1, f, a2, 0:DH],
                                ps[0:1, 0, 128 * a2:128 * a2 + DH])
                            nc.vector.tensor_copy(
                                evp[32:33, f, a2, 0:DH],
                                ps[32:33, 0, 128 * a2 + DH:128 * a2 + 2 * DH])
                    return t

                def qk_chunk(f, cc, ih):
                    def t():
                        ps = psp.tile([128, 2, NI], F32, tag="sim", bufs=2)
                        for dc in range(NDC):
                            nc.tensor.matmul(
                                ps[:, 0],
                                wq[:, dc, cc * 128:(cc + 1) * 128],
                                xT[:, dc, f, ih * NI:(ih + 1) * NI],
                                start=(dc == 0), stop=(dc == NDC - 1),
                            )
                        if cc < 4:
                            nc.vector.tensor_copy(qT[:, cc, f, ih * NI:(ih + 1) * NI],
                                                  ps[:, 0])
                        else:
                            nc.vector.tensor_copy(kT[:, cc - 4, f, ih * NI:(ih + 1) * NI],
                                                  ps[:, 0])
                    return t

                def proj_chunk(f, ic):
                    def t():
                        ps = psp.tile([128, 2, NI], F32, tag="sim", bufs=2)
                        for a in range(NDC):
                            nc.tensor.matmul(
                                ps[:, 0],
                                attn[:, a, f, ic * 128:(ic + 1) * 128],
                                wo[:, a, :],
                                start=(a == 0), stop=(a == NDC - 1),
                            )
                        ot = wp.tile([128, D], BF16, tag="oout")
                        nc.vector.tensor_copy(ot[:], ps[:, 0])
                        nc.sync.dma_start(out_d[f, ic * 128:(ic + 1) * 128, :], ot[:])
                    return t

                # ---- the injection queue ----
                # entries: (due_slot, earliest_slot, thunk). A thunk is
                # force-emitted when its deadline nears; otherwise one
                # optional thunk runs every other slot to spread PE load.
                queue = []
                cur_slot = [0]

                def inject():
                    cur = cur_slot[0]
                    while queue and queue[0][0] <= cur + 3:
                        queue.pop(0)[2]()
                    if queue and queue[0][1] <= cur:
                        queue.pop(0)[2]()

                def drain():
                    while queue:
                        queue.pop(0)[2]()

                def emit_block(f, a, ih, epi_prev=None):
                    isl = slice(ih * NI, (ih + 1) * NI)
                    pvA = psp.tile([65, NI], F32, tag="pv", bufs=4)
                    pvB = psp.tile([65, NI], F32, tag="pv", bufs=4)
                    # software-pipelined: sims run ahead of pvs so a pv
                    # waiting on its exp never head-of-line-blocks the
                    # next sims in the PE queue
                    pABs = [None] * JC
                    pc = wp.tile([33, NI], BF16, tag="pc", bufs=4)

                    def emit_cond_sim():
                        # block-diag stationary: rows 0/32 = cond logits A/B
                        psc = psp.tile([128, 2, NI], F32, tag="sim", bufs=2)
                        nc.tensor.matmul(
                            psc[0:33, 0], ekk[:, f, a, 0:33], qT[:, a, f, isl],
                            start=True, stop=True,
                        )
                        nc.scalar.activation(pc[:], psc[0:33, 0], EXP, scale=SCALE)

                    def emit_cond_pv():
                        # rank-1 cond update [ev_h | 1] x pc row; closes accum
                        nc.tensor.matmul(
                            pvA[:], evp[0:1, f, a, 0:65], pc[0:1, :],
                            start=False, stop=True,
                        )
                        nc.tensor.matmul(
                            pvB[:], evp[32:33, f, a, 0:65], pc[32:33, :],
                            start=False, stop=True,
                        )

                    def emit_sim(jc):
                        jsl = slice(jc * 128, (jc + 1) * 128)
                        sAB = psp.tile([128, 2, NI], F32, tag="sim", bufs=2)
                        nc.tensor.matmul(
                            sAB[:, 0], kT[0:64, a, f, jsl], qT[0:64, a, f, isl],
                            start=True, stop=True, tile_position=(0, 0),
                        )
                        nc.tensor.matmul(
                            sAB[:, 1], kT[64:128, a, f, jsl], qT[64:128, a, f, isl],
                            start=True, stop=True, tile_position=(64, 0),
                        )
                        # one exp covering both heads' chunks (2 PSUM banks)
                        pAB = wp.tile([128, 2, NI], BF16, tag="P")
                        nc.scalar.activation(pAB[:], sAB[:], EXP, scale=SCALE)
                        pABs[jc] = pAB

                    def emit_pv(jc):
                        nc.tensor.matmul(
                            pvA[:], vv[:, f, jc, 2 * a, 0:65], pABs[jc][:, 0],
                            start=(jc == 0), stop=False,
                        )
                        nc.tensor.matmul(
                            pvB[:], vv[:, f, jc, 2 * a + 1, 0:65], pABs[jc][:, 1],
                            start=(jc == 0), stop=False,
                        )

                    for jc in range(JC):
                        emit_sim(jc)
                        if jc <= 1:
                            # previous block's tail (last pv, cond-pv) spills
                            # here so it never blocks our first sims
                            if pv_spill:
                                pv_spill.pop(0)()
                        else:
                            emit_pv(jc - 2)
                        if jc == 2 and epi_prev is not None:
                            # previous block's reciprocals land on DVE now,
                            # their latency hidden behind this block's sims
                            epi_prev[0]()
                        elif jc == 4 and epi_prev is not None:
                            epi_prev[1]()
                        elif jc == 5:
                            emit_cond_sim()
                        inject()
                        cur_slot[0] += 1
                    emit_pv(JC - 2)
                    pv_spill.append(lambda: emit_pv(JC - 1))
                    pv_spill.append(emit_cond_pv)

                    def epi_recip():
                        # reciprocal softmax denominators (dummies contribute 0)
                        with nc.allow_low_precision("softmax denom recip bf16"):
                            nc.vector.reciprocal(rg[0:1, :], pvA[64:65, :])
                            nc.vector.reciprocal(rg[32:33, :], pvB[64:65, :])

                    def epi_norm():
                        # broadcast 1/denom: h1 -> partitions 0:64, h2 -> 64:128
                        bc = psp.tile([128, 2, NI], F32, tag="sim", bufs=2)
                        nc.tensor.matmul(bc[:, 0], fmat[:], rg[:], start=True, stop=True)
                        rbc = wp.tile([128, NI], BF16, tag="rbc")
                        nc.vector.tensor_copy(rbc[:], bc[:, 0])
                        nc.vector.tensor_mul(attn[0:64, a, f, isl], pvA[0:64, :], rbc[0:64, :])
                        nc.vector.tensor_mul(attn[64:128, a, f, isl], pvB[0:64, :], rbc[64:128, :])

                    return (epi_recip, epi_norm)

                # ---- schedule ----
                # prologue: minimum to start block (0, 0, 0)
                qk_chunk(0, 0, 0)(); qk_chunk(0, 4, 0)()
                v_chunk(0, 0)(); v_chunk(0, 1)()

                def slot_of(bi, jc):
                    return bi * JC + jc

                def fill_frame_queue(f):
                    b0 = 8 * f
                    ents = []
                    # v chunks: consumer pv(i) of block b0 at slot b0*9+i+1
                    for i in range(2 if f == 0 else 0, NTC):
                        ents.append((slot_of(b0, max(i - 1, 0)), 0, v_chunk(f, i)))
                    # k chunks: sims jc in [4ih, 4ih+3] of pair blocks
                    # q chunks: sims of block (f, p, ih)
                    for p in range(1, 4):
                        for ih in range(NIH):
                            ents.append((slot_of(b0 + 2 * p, 4 * ih) - 1, 0,
                                         qk_chunk(f, p + 4, ih)))
                            ents.append((slot_of(b0 + 2 * p + ih, 0) - 1, 0,
                                         qk_chunk(f, p, ih)))
                    if f == 0:
                        # pair-0 second halves (ih=1) not covered by prologue
                        ents.append((slot_of(0, 4) - 1, 0, qk_chunk(0, 4, 1)))
                        ents.append((slot_of(1, 0) - 1, 0, qk_chunk(0, 0, 1)))
                        ents.append((slot_of(0, 4), 0, ekk_thunk()))
                        # frame-1 head start (pair 0 q/k)
                        for ih in range(NIH):
                            ents.append((slot_of(8, 4 * ih) - 1, 0,
                                         qk_chunk(1, 4, ih)))
                            ents.append((slot_of(8 + ih, 0) - 1, 0,
                                         qk_chunk(1, 0, ih)))
                    ents.append((slot_of(b0 + 1, 0) - 1, 0, evp_thunk(f)))
                    if f == 1:
                        # frame-0 out-projection: attn(f=0) complete after
                        # block-7's epilogue, which runs early in block 8
                        for ic in range(NTC):
                            ents.append((slot_of(15, 7), slot_of(8, 5) + 2 * ic,
                                         proj_chunk(0, ic)))
                    queue.extend(sorted(ents, key=lambda e: e[0]))

                fill_frame_queue(0)
                blocks = [(f, a, ih) for f in range(2) for a in range(4)
                          for ih in range(NIH)]
                epi = None
                pv_spill = []
                for bi, (f, a, ih) in enumerate(blocks):
                    if bi == 4:
                        fill_frame_queue(1)
                    if bi == 15:
                        # frame-1 ih=0 projection: deps ready once block-14's
                        # epilogue (emitted at jc=4 of this block) is done
                        queue.extend((slot_of(15, 7), slot_of(15, 5) + ic, proj_chunk(1, ic))
                                     for ic in range(NTC // 2))
                    epi = emit_block(f, a, ih, epi_prev=epi)
                while pv_spill:
                    pv_spill.pop(0)()
                epi[0](); epi[1]()
                drain()
                # epilogue: frame-1 second-half projection
                for ic in range(NTC // 2, NTC):
                    proj_chunk(1, ic)()

            if loop_n > 1:
                with tc.For_i(0, loop_n, 1):
                    emit_body()
            else:
                emit_body()

    nc.finalize()
    return nc


_NC_CACHE = {}


def _get_nc(T):
    if T not in _NC_CACHE:
        _NC_CACHE[T] = build_attention_nc(T)
    return _NC_CACHE[T]


def make_in_maps(x, label_emb_mm, Wqkv, Wk, Wv, Wout):
    """Host-side sharding + layout prep (transpose to feature-major, bf16)."""
    bf = ml_dtypes.bfloat16
    BN, T, d = x.shape
    assert (BN, d) == (16, D)
    # x[fr, t, dc*128+p] -> xB[fr, p, dc, t]
    xB = np.ascontiguousarray(
        np.asarray(x).reshape(16, T, NDC, 128).transpose(0, 3, 2, 1)
    ).astype(bf)
    wq = np.ascontiguousarray(np.asarray(Wqkv).reshape(NDC, 128, 3 * HID).transpose(1, 0, 2)).astype(bf)
    wkh = np.ascontiguousarray(np.asarray(Wk).reshape(NDC, 128, HID).transpose(1, 0, 2)).astype(bf)
    wvh = np.ascontiguousarray(np.asarray(Wv).reshape(NDC, 128, HID).transpose(1, 0, 2)).astype(bf)
    woh = np.ascontiguousarray(np.asarray(Wout).reshape(NDC, 128, D).transpose(1, 0, 2)).astype(bf)
    labB = np.asarray(label_emb_mm).reshape(16, NDC, 128)  # [fr, dc, p]
    F = np.zeros((33, 128), dtype=bf)
    F[0, 0:64] = 1.0
    F[32, 64:128] = 1.0
    in_maps = []
    for c in range(N_CORES):
        xTc = np.ascontiguousarray(xB[2 * c:2 * c + 2].transpose(1, 2, 0, 3))  # (128,4,2,T)
        labc2 = np.ascontiguousarray(labB[2 * c:2 * c + 2].transpose(2, 1, 0)).astype(bf)  # (128,4,2)
        labc = np.zeros((128, NDC, 2, 40), dtype=bf)  # 16B-aligned f-stride
        labc[:, :, :, 0] = labc2
        labc[:, :, :, 32] = labc2  # duplicate for the ev row-32 stationary
        in_maps.append({
            "xT": xTc, "Wqkv": wq, "Wk": wkh, "Wv": wvh, "Wout": woh, "labT": labc,
            "F": F,
        })
    return in_maps


def kernel(x, label_emb_mm, Wqkv, Wk, Wv, Wout, b):
    x = np.asarray(x)
    T = x.shape[1]
    nc = _get_nc(T)
    in_maps = make_in_maps(x, label_emb_mm, Wqkv, Wk, Wv, Wout)
    res = run_bass_kernel_spmd(nc, in_maps, core_ids=list(range(N_CORES)))
    out = np.concatenate([res.results[c]["out"] for c in range(N_CORES)], axis=0)
    return np.ascontiguousarray(out.reshape(16, T, D)).astype(np.float32)


# revision 4
# speedup vs baseline: 1.2972x; 1.0482x over previous
"""Trainium2 Bass kernel: batched multi-head attention with per-frame
conditioning K/V token (nn_Attention dense_transformer problem).

Data-parallel over the 16 (b*n) frames -> 2 frames per NeuronCore, no
collectives. Per core, a fused kernel: QKV projection (q,k feature-major;
v token-major) -> per-head attention with sim computed transposed (keys on
partitions) so softmax denominators come from a ones-column in the PV
matmul -> output projection.

Scheduling (the main speedup over v1): software-pipelined emission.
Attention blocks (sim -> exp -> pv) are the scalar-engine-feeding backbone;
all other PE work (QKV projection chunks, v emission, cond-token k/v,
out-projection) is queued as thunks with deadlines and injected between jc
iterations, so ACT never starves and PE never bursts. Within a block, sims
run two jc ahead of pvs (a pv waiting on its exp would head-of-line-block
the PE wait queue, depth 4); the last pv and the cond-token pv spill into
the next block; the reciprocal/normalize epilogue is deferred into the next
block so its DVE latency hides behind sims. One exp per (pair, jc) covers
both heads via a 2-bank PSUM tile. Input DMAs are strided and ordered by
first use; output is DMA'd as bf16 (host casts back to f32).

Conditioning token: instead of padding keys to T+128 (which costs a full
extra key-chunk of sim/exp/pv per block), the cond token runs through its
own block-diagonal path: ekk holds [ekA | ekB] key columns at rows 0/32 of
a (128, 33) stationary, giving both heads' cond logits in one matmul; a
small (33, NI) exp produces pc; and a rank-1 [ev_h | 1] x pc matmul closes
each pv accumulation (the ones entry adds exp(cond) to the denominator).

Layout notes:
 - All matmul operands bf16 (f32 PSUM accumulation). Host pre-transposes x
   to feature-major and pre-splits d into 128-row chunks, so no on-device
   transposes are needed anywhere.
 - v stored interleaved [8 heads x 72 cols] (64 v + ones-col at 64 + 7 pad)
   so each head's PV stationary operand is a contiguous (128, 65) slice at
   a 16B-aligned offset (HW weight requirement); PV output row 64
   accumulates the softmax denominator for free.
"""

import numpy as np
import ml_dtypes

import concourse.bacc as bacc
import concourse.tile as tile
from concourse import mybir
from concourse.bass_utils import run_bass_kernel_spmd

BF16 = mybir.dt.bfloat16
F32 = mybir.dt.float32

HEADS = 8
DH = 64
D = 512
HID = 512
SCALE = DH ** -0.5
N_CORES = 8
NDC = D // 128  # 4 contraction chunks of 128


def build_attention_nc(T=1024, loop_n=1):
    S = T                   # keys only; cond token handled separately
    JC = S // 128           # key chunks (8 for T=1024)
    NI = min(512, T)        # i-tile width (matmul moving free dim)
    NIH = T // NI           # i-tiles per frame
    NTC = T // 128          # token chunks (for v / out-proj)

    nc = bacc.Bacc("TRN2", target_bir_lowering=False)
    x_d = nc.declare_dram_parameter("xT", [128, NDC, 2, T], BF16, isOutput=False)
    w_d = nc.declare_dram_parameter("Wqkv", [128, NDC, 3 * HID], BF16, isOutput=False)
    wk_d = nc.declare_dram_parameter("Wk", [128, NDC, HID], BF16, isOutput=False)
    wv_d = nc.declare_dram_parameter("Wv", [128, NDC, HID], BF16, isOutput=False)
    wo_d = nc.declare_dram_parameter("Wout", [128, NDC, D], BF16, isOutput=False)
    lab_d = nc.declare_dram_parameter("labT", [128, NDC, 2, 40], BF16, isOutput=False)
    f_d = nc.declare_dram_parameter("F", [33, 128], BF16, isOutput=False)
    out_d = nc.declare_dram_parameter("out", [2, T, D], BF16, isOutput=True)

    EXP = mybir.ActivationFunctionType.Exp

    with tile.TileContext(nc) as tc:
        with (
            tc.tile_pool(name="persist", bufs=1) as pp,
            tc.tile_pool(name="work", bufs=8) as wp,
            tc.tile_pool(name="psum", bufs=1, space="PSUM") as psp,
        ):
            def emit_body():
                # ---- persistent SBUF tiles ----
                xT = pp.tile([128, NDC, 2, T], BF16, tag="xT")
                wq = pp.tile([128, NDC, 3 * HID], BF16, tag="wq")
                wk = pp.tile([128, NDC, HID], BF16, tag="wk")
                wv = pp.tile([128, NDC, HID], BF16, tag="wv")
                wo = pp.tile([128, NDC, D], BF16, tag="wo")
                lab = pp.tile([128, NDC, 2, 40], BF16, tag="lab")
                qT = pp.tile([128, NDC, 2, T], BF16, tag="qT")
                kT = pp.tile([128, NDC, 2, S], BF16, tag="kT")
                # 72*2B = 144B: 16B-aligned per-head stride (HW weight req)
                vv = pp.tile([128, 2, JC, HEADS, 72], BF16, tag="vv")
                # cond-token stationaries: ekk block-diag key cols (rows 0/32)
                # per (f, pair); evp rows [ev_h | 1] for the cond rank-1 pv
                ekk = pp.tile([128, 2, 4, 40], BF16, tag="ekk")
                evp = pp.tile([33, 2, 4, 72], BF16, tag="evp")
                attn = pp.tile([128, NDC, 2, T], BF16, tag="attn")
                fmat = pp.tile([33, 128], BF16, tag="fmat")
                # 1/denom rows: 0 (h1) and 32 (h2); rows 1-31 stay 1.0
                rg = pp.tile([33, NI], BF16, tag="rg")

                # ---- input DMAs, strided, ordered by first use ----
                nc.sync.dma_start(wq[:, :, 0:128], w_d[:, :, 0:128])      # q cc0
                nc.sync.dma_start(wq[:, :, 512:640], w_d[:, :, 512:640])  # k cc4
                nc.sync.dma_start(xT[:, :, 0, 0:NI], x_d[:, :, 0, 0:NI])
                nc.sync.dma_start(wq[:, :, 2 * HID:3 * HID],              # v cols
                                  w_d[:, :, 2 * HID:3 * HID])
                nc.sync.dma_start(xT[:, :, 0, NI:T], x_d[:, :, 0, NI:T])
                nc.sync.dma_start(wq[:, :, 128:512], w_d[:, :, 128:512])
                nc.sync.dma_start(wq[:, :, 640:1024], w_d[:, :, 640:1024])
                nc.sync.dma_start(lab[:], lab_d[:])
                nc.sync.dma_start(wk[:], wk_d[:])
                nc.sync.dma_start(wv[:], wv_d[:])
                nc.sync.dma_start(fmat[:], f_d[:])
                for dc in range(NDC):  # frame-1 activations
                    nc.sync.dma_start(xT[:, dc, 1], x_d[:, dc, 1])
                nc.sync.dma_start(wo[:], wo_d[:])

                # constants init (rg rows 1-31 finite; F rows 0 there)
                nc.vector.memset(rg[:], 1.0)
                nc.vector.memset(vv[:, :, :, :, DH:DH + 1], 1.0)  # ones col
                nc.vector.memset(ekk[:], 0.0)
                nc.vector.memset(evp[:, :, :, DH:DH + 1], 1.0)  # cond ones

                # HAM warm-up: the PE clock gate sits at 1.2 GHz until ~3.4us
                # of sustained activity. The first real matmuls wait on input
                # DMA anyway, so burn that window on dummy matmuls (reading
                # the just-memset rg) to enter the first block at 2.4 GHz.
                warm = psp.tile([128, 2, NI], F32, tag="sim", bufs=2)
                for _ in range(8):
                    nc.tensor.matmul(warm[:, 0], rg[:, 0:128], rg[:, 0:NI],
                                     start=True, stop=True)

                # ---- emission helpers (each returns a list of thunks) ----
                def v_chunk(f, tc_i):
                    def t():
                        ps = psp.tile([128, 2, NI], F32, tag="sim", bufs=2)
                        for dc in range(NDC):
                            nc.tensor.matmul(
                                ps[:, 0, 0:HID],
                                xT[:, dc, f, tc_i * 128:(tc_i + 1) * 128],
                                wq[:, dc, 2 * HID:3 * HID],
                                start=(dc == 0), stop=(dc == NDC - 1),
                            )
                        nc.vector.tensor_copy(vv[:, f, tc_i, :, 0:DH], ps[:, 0, 0:HID])
                    return t

                def ekk_thunk():
                    def t():
                        # ek per pair: (128 feats, 2 frames) -> block-diag
                        # stationaries: A at col 0 rows 0:64, B col 32 rows 64:128
                        for cc in range(NDC):
                            ps = psp.tile([128, 2, NI], F32, tag="sim", bufs=2)
                            for dc in range(NDC):
                                nc.tensor.matmul(
                                    ps[:, 0, 0:2],
                                    wk[:, dc, cc * 128:(cc + 1) * 128],
                                    lab[:, dc, :, 0:1],
                                    start=(dc == 0), stop=(dc == NDC - 1),
                                )
                            for f in range(2):
                                nc.vector.tensor_copy(
                                    ekk[0:64, f, cc, 0:1], ps[0:64, 0, f:f + 1])
                                nc.vector.tensor_copy(
                                    ekk[64:128, f, cc, 32:33], ps[64:128, 0, f:f + 1])
                    return t

                def evp_thunk(f):
                    def t():
                        # ev on partitions 0 AND 32 (lab col duplicated on host)
                        ps = psp.tile([128, 2, NI], F32, tag="sim", bufs=2)
                        for dc in range(NDC):
                            nc.tensor.matmul(
                                ps[0:33, 0, 0:HID],
                                lab[:, dc, f, 0:33],
                                wv[:, dc, :],
                                start=(dc == 0), stop=(dc == NDC - 1),
                            )
                        for a2 in range(4):
                            nc.vector.tensor_copy(
                                evp[0:1, f, a2, 0:DH],
                                ps[0:1, 0, 128 * a2:128 * a2 + DH])
                            nc.vector.tensor_copy(
                                evp[32:33, f, a2, 0:DH],
                                ps[32:33, 0, 128 * a2 + DH:128 * a2 + 2 * DH])
                    return t

                def qk_chunk(f, cc, ih):
                    def t():
                        ps = psp.tile([128, 2, NI], F32, tag="sim", bufs=2)
                        for dc in range(NDC):
                            nc.tensor.matmul(
                                ps[:, 0],
                                wq[:, dc, cc * 128:(cc + 1) * 128],
                                xT[:, dc, f, ih * NI:(ih + 1) * NI],
                                start=(dc == 0), stop=(dc == NDC - 1),
                            )
                        if cc < 4:
                            nc.vector.tensor_copy(qT[:, cc, f, ih * NI:(ih + 1) * NI],
                                                  ps[:, 0])
                        else:
                            nc.vector.tensor_copy(kT[:, cc - 4, f, ih * NI:(ih + 1) * NI],
                                                  ps[:, 0])
                    return t

                def proj_chunk(f, ic):
                    def t():
                        ps = psp.tile([128, 2, NI], F32, tag="sim", bufs=2)
                        for a in range(NDC):
                            nc.tensor.matmul(
                                ps[:, 0],
                                attn[:, a, f, ic * 128:(ic + 1) * 128],
                                wo[:, a, :],
                                start=(a == 0), stop=(a == NDC - 1),
                            )
                        ot = wp.tile([128, D], BF16, tag="oout")
                        nc.vector.tensor_copy(ot[:], ps[:, 0])
                        nc.sync.dma_start(out_d[f, ic * 128:(ic + 1) * 128, :], ot[:])
                    return t

                # ---- the injection queue ----
                # entries: (due_slot, earliest_slot, thunk). A thunk is
                # force-emitted when its deadline nears; otherwise one
                # optional thunk runs every other slot to spread PE load.
                queue = []
                cur_slot = [0]

                def inject():
                    cur = cur_slot[0]
                    while queue and queue[0][0] <= cur + 3:
                        queue.pop(0)[2]()
                    if queue and queue[0][1] <= cur:
                        queue.pop(0)[2]()

                def drain():
                    while queue:
                        queue.pop(0)[2]()

                def emit_block(f, a, ih, epi_prev=None):
                    isl = slice(ih * NI, (ih + 1) * NI)
                    pvA = psp.tile([65, NI], F32, tag="pv", bufs=4)
                    pvB = psp.tile([65, NI], F32, tag="pv", bufs=4)
                    # software-pipelined: sims run ahead of pvs so a pv
                    # waiting on its exp never head-of-line-blocks the
                    # next sims in the PE queue
                    pABs = [None] * JC
                    pc = wp.tile([33, NI], BF16, tag="pc", bufs=4)

                    def emit_cond_sim():
                        # block-diag stationary: rows 0/32 = cond logits A/B
                        psc = psp.tile([128, 2, NI], F32, tag="sim", bufs=2)
                        nc.tensor.matmul(
                            psc[0:33, 0], ekk[:, f, a, 0:33], qT[:, a, f, isl],
                            start=True, stop=True,
                        )
                        nc.scalar.activation(pc[:], psc[0:33, 0], EXP, scale=SCALE)

                    def emit_cond_pv():
                        # rank-1 cond update [ev_h | 1] x pc row; closes accum
                        nc.tensor.matmul(
                            pvA[:], evp[0:1, f, a, 0:65], pc[0:1, :],
                            start=False, stop=True,
                        )
                        nc.tensor.matmul(
                            pvB[:], evp[32:33, f, a, 0:65], pc[32:33, :],
                            start=False, stop=True,
                        )

                    def emit_sim(jc):
                        jsl = slice(jc * 128, (jc + 1) * 128)
                        sAB = psp.tile([128, 2, NI], F32, tag="sim", bufs=2)
                        nc.tensor.matmul(
                            sAB[:, 0], kT[0:64, a, f, jsl], qT[0:64, a, f, isl],
                            start=True, stop=True, tile_position=(0, 0),
                        )
                        nc.tensor.matmul(
                            sAB[:, 1], kT[64:128, a, f, jsl], qT[64:128, a, f, isl],
                            start=True, stop=True, tile_position=(64, 0),
                        )
                        # one exp covering both heads' chunks (2 PSUM banks)
                        pAB = wp.tile([128, 2, NI], BF16, tag="P")
                        nc.scalar.activation(pAB[:], sAB[:], EXP, scale=SCALE)
                        pABs[jc] = pAB

                    def emit_pv(jc):
                        nc.tensor.matmul(
                            pvA[:], vv[:, f, jc, 2 * a, 0:65], pABs[jc][:, 0],
                            start=(jc == 0), stop=False,
                        )
                        nc.tensor.matmul(
                            pvB[:], vv[:, f, jc, 2 * a + 1, 0:65], pABs[jc][:, 1],
                            start=(jc == 0), stop=False,
                        )

                    for jc in range(JC):
                        emit_sim(jc)
                        if jc <= 1:
                            # previous block's tail (last pv, cond-pv) spills
                            # here so it never blocks our first sims
                            if pv_spill:
                                pv_spill.pop(0)()
                        else:
                            emit_pv(jc - 2)
                        if jc == 2 and epi_prev is not None:
                            # previous block's reciprocals land on DVE now,
                            # their latency hidden behind this block's sims
                            epi_prev[0]()
                        elif jc == 4 and epi_prev is not None:
                            epi_prev[1]()
                        elif jc == 5:
                            emit_cond_sim()
                        inject()
                        cur_slot[0] += 1
                    emit_pv(JC - 2)
                    pv_spill.append(lambda: emit_pv(JC - 1))
                    pv_spill.append(emit_cond_pv)

                    def epi_recip():
                        # reciprocal softmax denominators (dummies contribute 0)
                        with nc.allow_low_precision("softmax denom recip bf16"):
                            nc.vector.reciprocal(rg[0:1, :], pvA[64:65, :])
                            nc.vector.reciprocal(rg[32:33, :], pvB[64:65, :])

                    def epi_norm():
                        # broadcast 1/denom: h1 -> partitions 0:64, h2 -> 64:128
                        bc = psp.tile([128, 2, NI], F32, tag="sim", bufs=2)
                        nc.tensor.matmul(bc[:, 0], fmat[:], rg[:], start=True, stop=True)
                        rbc = wp.tile([128, NI], BF16, tag="rbc")
                        nc.vector.tensor_copy(rbc[:], bc[:, 0])
                        nc.vector.tensor_mul(attn[0:64, a, f, isl], pvA[0:64, :], rbc[0:64, :])
                        nc.vector.tensor_mul(attn[64:128, a, f, isl], pvB[0:64, :], rbc[64:128, :])

                    return (epi_recip, epi_norm)

                # ---- schedule ----
                # prologue: minimum to start block (0, 0, 0)
                qk_chunk(0, 0, 0)(); qk_chunk(0, 4, 0)()
                v_chunk(0, 0)(); v_chunk(0, 1)()

                def slot_of(bi, jc):
                    return bi * JC + jc

                def fill_frame_queue(f):
                    b0 = 8 * f
                    ents = []
                    # v chunks: consumer pv(i) of block b0 at slot b0*9+i+1
                    for i in range(2 if f == 0 else 0, NTC):
                        ents.append((slot_of(b0, max(i - 1, 0)), 0, v_chunk(f, i)))
                    # k chunks: sims jc in [4ih, 4ih+3] of pair blocks
                    # q chunks: sims of block (f, p, ih)
                    for p in range(1, 4):
                        for ih in range(NIH):
                            ents.append((slot_of(b0 + 2 * p, 4 * ih) - 1, 0,
                                         qk_chunk(f, p + 4, ih)))
                            ents.append((slot_of(b0 + 2 * p + ih, 0) - 1, 0,
                                         qk_chunk(f, p, ih)))
                    if f == 0:
                        # pair-0 second halves (ih=1) not covered by prologue
                        ents.append((slot_of(0, 4) - 1, 0, qk_chunk(0, 4, 1)))
                        ents.append((slot_of(1, 0) - 1, 0, qk_chunk(0, 0, 1)))
                        ents.append((slot_of(0, 4), 0, ekk_thunk()))
                        # frame-1 head start (pair 0 q/k)
                        for ih in range(NIH):
                            ents.append((slot_of(8, 4 * ih) - 1, 0,
                                         qk_chunk(1, 4, ih)))
                            ents.append((slot_of(8 + ih, 0) - 1, 0,
                                         qk_chunk(1, 0, ih)))
                    ents.append((slot_of(b0 + 1, 0) - 1, 0, evp_thunk(f)))
                    if f == 1:
                        # frame-0 out-projection: attn(f=0) complete after
                        # block-7's epilogue, which runs early in block 8
                        for ic in range(NTC):
                            ents.append((slot_of(15, 7), slot_of(8, 5) + 2 * ic,
                                         proj_chunk(0, ic)))
                    queue.extend(sorted(ents, key=lambda e: e[0]))

                fill_frame_queue(0)
                blocks = [(f, a, ih) for f in range(2) for a in range(4)
                          for ih in range(NIH)]
                epi = None
                pv_spill = []
                for bi, (f, a, ih) in enumerate(blocks):
                    if bi == 4:
                        fill_frame_queue(1)
                    if bi == 15:
                        # frame-1 ih=0 projection: deps ready once block-14's
                        # epilogue (emitted at jc=4 of this block) is done
                        queue.extend((slot_of(15, 7), slot_of(15, 5) + ic, proj_chunk(1, ic))
                                     for ic in range(NTC // 2))
                    epi = emit_block(f, a, ih, epi_prev=epi)
                while pv_spill:
                    pv_spill.pop(0)()
                epi[0](); epi[1]()
                drain()
                # epilogue: frame-1 second-half projection
                for ic in range(NTC // 2, NTC):
                    proj_chunk(1, ic)()

            if loop_n > 1:
                with tc.For_i(0, loop_n, 1):
                    emit_body()
            else:
                emit_body()

    nc.finalize()
    return nc


_NC_CACHE = {}


def _get_nc(T):
    if T not in _NC_CACHE:
        _NC_CACHE[T] = build_attention_nc(T)
    return _NC_CACHE[T]


def make_in_maps(x, label_emb_mm, Wqkv, Wk, Wv, Wout):
    """Host-side sharding + layout prep (transpose to feature-major, bf16)."""
    bf = ml_dtypes.bfloat16
    BN, T, d = x.shape
    assert (BN, d) == (16, D)
    # x[fr, t, dc*128+p] -> xB[fr, p, dc, t]
    xB = np.ascontiguousarray(
        np.asarray(x).reshape(16, T, NDC, 128).transpose(0, 3, 2, 1)
    ).astype(bf)
    wq = np.ascontiguousarray(np.asarray(Wqkv).reshape(NDC, 128, 3 * HID).transpose(1, 0, 2)).astype(bf)
    wkh = np.ascontiguousarray(np.asarray(Wk).reshape(NDC, 128, HID).transpose(1, 0, 2)).astype(bf)
    wvh = np.ascontiguousarray(np.asarray(Wv).reshape(NDC, 128, HID).transpose(1, 0, 2)).astype(bf)
    woh = np.ascontiguousarray(np.asarray(Wout).reshape(NDC, 128, D).transpose(1, 0, 2)).astype(bf)
    labB = np.asarray(label_emb_mm).reshape(16, NDC, 128)  # [fr, dc, p]
    F = np.zeros((33, 128), dtype=bf)
    F[0, 0:64] = 1.0
    F[32, 64:128] = 1.0
    in_maps = []
    for c in range(N_CORES):
        xTc = np.ascontiguousarray(xB[2 * c:2 * c + 2].transpose(1, 2, 0, 3))  # (128,4,2,T)
        labc2 = np.ascontiguousarray(labB[2 * c:2 * c + 2].transpose(2, 1, 0)).astype(bf)  # (128,4,2)
        labc = np.zeros((128, NDC, 2, 40), dtype=bf)  # 16B-aligned f-stride
        labc[:, :, :, 0] = labc2
        labc[:, :, :, 32] = labc2  # duplicate for the ev row-32 stationary
        in_maps.append({
            "xT": xTc, "Wqkv": wq, "Wk": wkh, "Wv": wvh, "Wout": woh, "labT": labc,
            "F": F,
        })
    return in_maps


def kernel(x, label_emb_mm, Wqkv, Wk, Wv, Wout, b):
    x = np.asarray(x)
    T = x.shape[1]
    nc = _get_nc(T)
    in_maps = make_in_maps(x, label_emb_mm, Wqkv, Wk, Wv, Wout)
    res = run_bass_kernel_spmd(nc, in_maps, core_ids=list(range(N_CORES)))
    out = np.concatenate([res.results[c]["out"] for c in range(N_CORES)], axis=0)
    return np.ascontiguousarray(out.reshape(16, T, D)).astype(np.float32)
